# revision 78
# baseline (speedup 1.0000x reference)
"""Trainium2 Bass kernel for MemoryEfficientFlashAttention (B=2,S=2048,HID=2048,H=16,HKV=8,D=128,CHUNK=512).

Sharding: 8 cores = 2 batches x 4 head-groups (4 q heads / 2 kv heads per core).
Each core computes q/k/v projections (+RoPE), the chunked flash-attention
recurrence, and a row-sharded partial of the output projection (transposed).
Host sums the 4 partials per batch and adds bo.

Math: the reference's scan step is algebraically
    o_j = (o_{j-1} * e^{m_{j-1}} + Y_j) / (e^{m_{j-1}} + S_j)
with Y_j = exp(sc_j - m_j) @ V_j, m_j the running max.  Unrolled:
    o_n = sum_j exp(sc_j - m_j) @ V_j * F_j,
    F_j = prod_{l>=j} d_l^-1 * d_n^-flag,   d_l = e^{m_{l-1}-m_l} + T_l,
where flag=1 iff the globally-last kv chunk was processed (the reference's
final o/d divide; skipped chunks have d=1 exactly under fp32 exp underflow).

Single score pass: scores are computed once ([q-part, k]) and exponentiated
with NO max subtraction (|sc| <~ 10, safe in fp32/bf16): P_t = exp(sc_t),
stored bf16, with T-hat accumulated by the activation.  Chunk maxes come
from bf16 reduces OFF the critical path, and the chains run on
M_t = e^{m_t} (running max of chunk maxes):
    d_t = (M_{t-1} + That_t) / M_t,     G_t = F_t / M_t
so no Ln/exp in the chain at all.  Pass 2 applies G and transposes P in one
PE op per 128-block:
    matmul(lhsT=P_block, rhs=diag(G))  ==  P^T diag(G)
and accumulates u = sum V^T (P^T diag(G)) directly in PSUM.  No score
recompute and no exponent-bias injection matmuls.  Engine placement per the
TRN2 ISA: PSUM evacuation only on DVE/Act, elementwise SBUF chain work on
GpSimd (which cannot touch PSUM or do tensor_tensor max), transposes/PV/
projections on PE.
"""

import os
import sys
from contextlib import ExitStack

import numpy as np
import ml_dtypes

sys.path.insert(0, "/opt/trn_rl_repo")
os.environ.setdefault("MYCRO_LOCAL_CACHE", "1")

import concourse.bass as bass  # noqa: E402
import concourse.tile as tile  # noqa: E402
from concourse import bacc, mybir  # noqa: E402
from concourse.bass_utils import run_bass_kernel_spmd  # noqa: E402

B, S, HID = 2, 2048, 2048
H, HKV, D = 16, 8, 128
CHUNK = 512
THETA = 1000000.0
NCORES = 8
HL = H // (NCORES // B)      # 4 local q heads
KVL = HKV // (NCORES // B)   # 2 local kv heads
NQ = S // CHUNK              # 4 chunks
NT = HID // 128              # 16 hid tiles
SCALE = 1.0 / np.sqrt(np.float32(D))
NEG = -1e9

F32 = mybir.dt.float32
F32R = mybir.dt.float32r
BF16 = mybir.dt.bfloat16
Alu = mybir.AluOpType
Act = mybir.ActivationFunctionType
AxX = mybir.AxisListType.X

_CACHE = {}


def _f32r_round(a):
    """Round fp32 to the fp32r format (1s/8e/11m in the high 20 bits):
    round-to-nearest-even at mantissa bit 12."""
    u = np.ascontiguousarray(a, dtype=np.float32).view(np.uint32).copy()
    low = u & np.uint32(0xFFF)
    base = u & ~np.uint32(0xFFF)
    lsb = (base >> 12) & np.uint32(1)
    round_up = (low > 0x800) | ((low == 0x800) & (lsb == 1))
    out = base + (round_up.astype(np.uint32) << 12)
    return out.view(np.float32)


def _bf16(a):
    return np.asarray(a, dtype=ml_dtypes.bfloat16)


def _rope_tables():
    inv_freq = 1.0 / (THETA ** (np.arange(0, D, 2, dtype=np.float32) / D))
    pos = np.arange(S, dtype=np.float32)
    freqs = pos[:, None].astype(np.float32) * inv_freq[None, :]
    emb = np.concatenate([freqs, freqs], axis=-1)  # [S, D]
    cosT = np.cos(emb).astype(np.float32).T.copy()
    sinT = np.sin(emb).astype(np.float32).T.copy()
    return cosT, sinT  # [D, S]


def _check_causal(am):
    """Assert the mask is the canonical causal mask (shared across batches)
    and return the per-qi chunk plan [(j, is_diag), ...]."""
    tri = np.tril(np.ones((S, S), dtype=bool))
    want = np.where(tri, 0.0, NEG).astype(np.float32)
    for b in range(am.shape[0]):
        assert np.array_equal(am[b, 0], want), "kernel specialized for causal mask"
    plan = {}
    for qi in range(NQ):
        plan[qi] = [(j, j == qi) for j in range(qi + 1)]
    return plan


class _Rot:
    """Round-robin engine selector for PSUM->SBUF copies."""

    def __init__(self, nc, engines):
        self.ops = []
        for e in engines:
            if e == "v":
                self.ops.append(lambda o, i: nc.vector.tensor_copy(o, i))
            elif e == "g":
                self.ops.append(lambda o, i: nc.gpsimd.tensor_copy(o, i))
            else:
                self.ops.append(lambda o, i: nc.scalar.copy(o, i))
        self.i = 0

    def copy(self, out, in_):
        self.ops[self.i % len(self.ops)](out, in_)
        self.i += 1


def _emit(tc, ap, plan):
    nc = tc.nc

    def mm(out, lhsT, rhs, start, stop):
        nc.tensor.matmul(out, lhsT, rhs, start=start, stop=stop)

    with ExitStack() as top:
        # ---------------- persistent tensors ----------------
        pers = top.enter_context(tc.tile_pool(name="pers", bufs=1))
        QT = pers.tile([128, HL, S], F32R)             # rope'd q^T  [d, h, s]
        KT = pers.tile([128, KVL, S], F32R)            # rope'd k^T  [d, kv, s]
        V = pers.tile([128, S // 128, KVL * D], BF16)  # v natural [s_p, s_t, kv*d]
        I128b = pers.tile([128, 128], BF16)
        maskcb = pers.tile([128, 4, CHUNK], BF16)      # causal diag-block mask rows
        ones1 = pers.tile([1, 128], F32R)
        R128 = pers.tile([128, 128], F32R)
        bqk = pers.tile([128, HL + KVL], F32)
        bv = pers.tile([1, KVL * D], F32R)

        # ---------------- phase A: projections + rope ----------------
        with ExitStack() as ph1:
            w_pool = ph1.enter_context(tc.tile_pool(name="wres", bufs=1))
            xt_pool = ph1.enter_context(tc.tile_pool(name="xt", bufs=2))
            cs_pool = ph1.enter_context(tc.tile_pool(name="cs", bufs=2))
            raw_pool = ph1.enter_context(tc.tile_pool(name="raw", bufs=2))
            t_pool = ph1.enter_context(tc.tile_pool(name="ropetmp", bufs=2))
            psP = ph1.enter_context(tc.tile_pool(name="psP", bufs=6, space="PSUM"))
            psR = ph1.enter_context(tc.tile_pool(name="psR", bufs=1, space="PSUM"))
            psV = ph1.enter_context(tc.tile_pool(name="psV", bufs=1, space="PSUM"))

            wqk_r = ap["wqk"].rearrange("(t p) m -> p t m", p=128)
            wqk_sb = w_pool.tile([128, NT, (HL + KVL) * D], BF16)
            wv_sb = w_pool.tile([128, NT, KVL * D], BF16)
            hsT_r = ap["hsT"].rearrange("(t p) s -> p t s", p=128)

            for sq in range(NQ):
                ssl = slice(sq * CHUNK, (sq + 1) * CHUNK)
                xt = xt_pool.tile([128, NT, CHUNK], BF16)
                cost = cs_pool.tile([128, CHUNK], F32, tag="cos")
                sint = cs_pool.tile([128, CHUNK], F32, tag="sin")
                if sq == 0:
                    # DMA order tuned so the first matmuls start ~1.5us in:
                    # weight/x tiles stream in consumption order, consts
                    # needed later go last.
                    nc.sync.dma_start(R128, ap["rmat"])
                    nc.sync.dma_start(bqk, ap["bqk"])
                    for t in range(4):
                        nc.sync.dma_start(wqk_sb[:, t], wqk_r[:, t])
                        nc.sync.dma_start(xt[:, t], hsT_r[:, t, ssl])
                    nc.sync.dma_start(cost, ap["cosT"][:, ssl])
                    nc.sync.dma_start(sint, ap["sinT"][:, ssl])
                    for t in range(4, NT):
                        nc.sync.dma_start(wqk_sb[:, t], wqk_r[:, t])
                        nc.sync.dma_start(xt[:, t], hsT_r[:, t, ssl])
                    nc.sync.dma_start(bv, ap["bv"])
                    nc.sync.dma_start(ones1, ap["ones1"])
                    nc.sync.dma_start(
                        wv_sb, ap["wv"].rearrange("(t p) m -> p t m", p=128))
                    nc.sync.dma_start(I128b, ap["imatb"])
                    nc.sync.dma_start(maskcb, ap["maskcb"])
                else:
                    for t in range(NT):
                        nc.sync.dma_start(xt[:, t], hsT_r[:, t, ssl])
                    nc.sync.dma_start(cost, ap["cosT"][:, ssl])
                    nc.sync.dma_start(sint, ap["sinT"][:, ssl])

                # q^T and k^T projections, rope'd.  sq==0 runs t-major so
                # each x tile is consumed as its DMA lands (6 open groups).
                if sq == 0:
                    pss = [psP.tile([128, CHUNK], F32, tag="sc", name=f"ps{m}")
                           for m in range(HL + KVL)]
                    for t in range(NT):
                        for m in range(HL + KVL):
                            mm(pss[m], wqk_sb[:, t, m * 128:(m + 1) * 128],
                               xt[:, t], start=(t == 0), stop=(t == NT - 1))
                for m in range(HL + KVL):
                    if sq == 0:
                        ps = pss[m]
                    else:
                        ps = psP.tile([128, CHUNK], F32, tag="sc", name="ps")
                        for t in range(NT):
                            mm(ps, wqk_sb[:, t, m * 128:(m + 1) * 128],
                               xt[:, t], start=(t == 0), stop=(t == NT - 1))
                    raw = raw_pool.tile([128, CHUNK], F32R)
                    nc.vector.tensor_scalar_add(raw, ps, bqk[:, m:m + 1])
                    pr = psR.tile([128, CHUNK], F32)
                    mm(pr, R128, raw, start=True, stop=True)
                    t1 = t_pool.tile([128, CHUNK], F32, tag="t1")
                    nc.gpsimd.tensor_mul(t1, raw.bitcast(F32), cost)
                    t2 = t_pool.tile([128, CHUNK], F32, tag="t2")
                    nc.vector.tensor_mul(t2, pr, sint)
                    dest = QT[:, m, ssl] if m < HL else KT[:, m - HL, ssl]
                    nc.vector.tensor_add(dest, t1, t2)

                # v projection (natural layout), bias via K=1 matmul
                for ss in range(CHUNK // 128):
                    pv = psV.tile([128, KVL * D], F32)
                    for t in range(NT):
                        mm(pv, xt[:, t, ss * 128:(ss + 1) * 128], wv_sb[:, t],
                           start=(t == 0), stop=False)
                    mm(pv, ones1, bv, start=False, stop=True)
                    (nc.vector.tensor_copy if ss % 2 else nc.scalar.copy)(V[:, sq * 4 + ss, :], pv)

        # ---------------- phase B: attention + out-proj ----------------
        with ExitStack() as ph2:
            pb_pool = ph2.enter_context(tc.tile_pool(name="probs", bufs=1))
            wo_pool = ph2.enter_context(tc.tile_pool(name="wo", bufs=1))
            ch_pool = ph2.enter_context(tc.tile_pool(name="chain", bufs=2))
            df_pool = ph2.enter_context(tc.tile_pool(name="dfp", bufs=12))
            pts_pool = ph2.enter_context(tc.tile_pool(name="pts", bufs=4))
            mh_pool = ph2.enter_context(tc.tile_pool(name="mh", bufs=2))
            o2_pool = ph2.enter_context(tc.tile_pool(name="uout", bufs=2))
            o_pool = ph2.enter_context(tc.tile_pool(name="osb", bufs=4))
            sc_ps = ph2.enter_context(tc.tile_pool(name="scps", bufs=3, space="PSUM"))
            u_ps = ph2.enter_context(tc.tile_pool(name="ups", bufs=1, space="PSUM"))
            misc_ps = ph2.enter_context(tc.tile_pool(name="mps", bufs=2, space="PSUM"))

            # bf16 probabilities, slot per (h, chunk-idx, q-sub)
            P = pb_pool.tile([128, HL, NQ, 4, CHUNK], BF16)
            wo_sb = wo_pool.tile([128, HL, HID], F32R)
            nc.sync.dma_start(wo_sb, ap["wo"].rearrange("(t p) m -> p t m", p=128))

            pt_rot = _Rot(nc, ["v", "s", "v"])
            drain_rot = _Rot(nc, ["s", "v"])
            ub_rot = _Rot(nc, ["v"])
            po_rot = _Rot(nc, ["s", "v"])

            # P_t = exp(sc_t) (no max subtraction: |sc| <~ 10 so exp is
            # safe in fp32/bf16).  Chunk maxes come from a single bf16
            # reduce per (h,t) OFF the critical path; the chains work on
            # M_t = e^{m_t} = running max of chunk maxes:
            #   d_t = (M_{t-1} + That_t) / M_t      (M_{-1} = 0)
            #   G_t = (prod_{l>=t} d_l^-1) * d_n^-flag / M_t
            # and the stored P gets weight P*G via diag(G) in pass 2.

            def p1_alloc(qi):
                tl = {n: ch_pool.tile([128, NQ, HL * 4], F32, tag=n, name=n)
                      for n in ("Ts", "Mr", "Sm", "rr", "Fw", "Gw")}
                tl["mx"] = ch_pool.tile([128, NQ, HL * 4, 8], F32, tag="mx",
                                        name="mx")
                return tl

            red_q = []   # deferred max-reduces: emitted one unit later so
                         # the DVE queue never head-of-line blocks on exps
            red_cnt = [0]

            def flush_reduce(n=1):
                for _ in range(min(n, len(red_q))):
                    red_q.pop(0)()

            def p1_unit(qi, t, h, tl):
                j, diag = plan[qi][t]
                narrow = diag and qi != 0
                for sub in range(4):
                    col = h * 4 + sub
                    w = (sub + 1) * 128 if narrow else CHUNK
                    q0 = qi * CHUNK + sub * 128
                    k0 = j * CHUNK
                    ps = sc_ps.tile([128, CHUNK], F32, tag="sc")
                    mm(ps[:, :w], QT[:, h, q0:q0 + 128],
                       KT[:, h // 2, k0:k0 + w],
                       start=True, stop=not diag)
                    if diag:
                        mm(ps[:, :w], I128b, maskcb[:, sub, :w],
                           start=False, stop=True)
                    nc.scalar.activation(
                        P[:, h, t, sub, :w], ps[:, :w], Act.Exp,
                        accum_out=tl["Ts"][:, t, col:col + 1])

                def reduce():
                    if narrow:
                        for sub in range(4):
                            col = h * 4 + sub
                            nc.vector.tensor_reduce(
                                tl["mx"][:, t, col, 0:1],
                                P[:, h, t, sub, :(sub + 1) * 128],
                                axis=AxX, op=Alu.max)
                    else:
                        # 2-level bf16 max tree on DVE (2x mode) + reduce
                        mh = mh_pool.tile([128, 4, CHUNK // 2], BF16,
                                          tag="mh")
                        nc.vector.tensor_tensor(
                            mh, P[:, h, t, :, :CHUNK // 2],
                            P[:, h, t, :, CHUNK // 2:], Alu.max)
                        mq = mh_pool.tile([128, 4, CHUNK // 4], BF16,
                                          tag="mq")
                        nc.vector.tensor_tensor(
                            mq, mh[:, :, :CHUNK // 4], mh[:, :, CHUNK // 4:],
                            Alu.max)
                        nc.vector.tensor_reduce(
                            tl["mx"][:, t, h * 4:h * 4 + 4, 0:1], mq,
                            axis=AxX, op=Alu.max)
                red_q.append(reduce)

            def p1_chains(qi, tl, cs=slice(0, HL * 4)):
                chunks = plan[qi]
                nj = len(chunks)
                Ts, mx, Mr, Sm, rr, Fw, Gw = (tl[n] for n in
                                              ("Ts", "mx", "Mr", "Sm", "rr",
                                               "Fw", "Gw"))
                nc.gpsimd.tensor_copy(Mr[:, 0, cs], mx[:, 0, cs, 0])
                for t in range(1, nj):
                    nc.vector.tensor_tensor(Mr[:, t, cs], Mr[:, t - 1, cs],
                                            mx[:, t, cs, 0], Alu.max)
                nc.gpsimd.tensor_copy(Sm[:, 0, cs], Ts[:, 0, cs])
                for t in range(1, nj):
                    nc.gpsimd.tensor_add(Sm[:, t, cs], Mr[:, t - 1, cs],
                                         Ts[:, t, cs])
                nc.vector.reciprocal(rr[:, :nj, cs], Sm[:, :nj, cs])
                nc.gpsimd.tensor_mul(rr[:, :nj, cs], rr[:, :nj, cs],
                                     Mr[:, :nj, cs])   # r_t = M_t / S_t
                if any(j == NQ - 1 for (j, _) in chunks):
                    nc.gpsimd.tensor_mul(rr[:, nj - 1, cs], rr[:, nj - 1, cs],
                                         rr[:, nj - 1, cs])
                nc.gpsimd.tensor_copy(Fw[:, nj - 1, cs], rr[:, nj - 1, cs])
                for t in range(nj - 2, -1, -1):
                    nc.gpsimd.tensor_mul(Fw[:, t, cs], Fw[:, t + 1, cs],
                                         rr[:, t, cs])
                nc.vector.reciprocal(Sm[:, :nj, cs], Mr[:, :nj, cs])
                nc.gpsimd.tensor_mul(Gw[:, :nj, cs], Fw[:, :nj, cs],
                                     Sm[:, :nj, cs])

            def build_dfs(Gw, t, h):
                dfs = []
                for sub in range(4):
                    df = df_pool.tile([128, 128], BF16, tag="df")
                    nc.gpsimd.tensor_scalar_mul(
                        df, I128b, Gw[:, t, h * 4 + sub:h * 4 + sub + 1])
                    dfs.append(df)
                return dfs

            def p2_unit(qi, t, h, ups, ubs, dfs, rot=None, utag="u",
                        ptag="pts"):
                # PV group per h: start at (t=0, kc=0) full-width; the diag
                # chunk (t=nj-1) runs kc DESC so the group's stop lands on a
                # full-width op (kc=0), keeping psum zero-region bookkeeping
                # clean.  qi==0's single chunk is full-width throughout.
                # The kc loop is software-pipelined: the next kc's transposes
                # are emitted before this kc's PV so PE never waits on the
                # psum->sbuf copy.
                rot = rot or pt_rot
                chunks = plan[qi]
                nj = len(chunks)
                j, diag = chunks[t]
                narrow = diag and qi != 0
                if t == 0:
                    ups[h] = u_ps.tile([128, CHUNK], F32, tag=utag, name="up")
                up = ups[h]

                def pv(kc, lo, pts):
                    mm(up[:, lo:],
                       V[:, j * 4 + kc, (h // 2) * D:(h // 2 + 1) * D],
                       pts[:, lo:], start=(t == 0 and kc == 0),
                       stop=(t == nj - 1 and kc == (0 if narrow else 3)))

                # kc pairs share one 2-bank psum tile and ONE evacuation
                # copy, halving copy-instruction count
                kprs = [(3, 2), (1, 0)] if narrow else [(0, 1), (2, 3)]
                pend = []

                def flush1():
                    pr, los, pt2, pts2 = pend.pop(0)
                    mlo = min(los)
                    rot.copy(pts2[:, :, mlo:], pt2[:, :, mlo:])
                    for half, kc in enumerate(pr):
                        pv(kc, los[half], pts2[:, half])

                for pr in kprs:
                    pt = misc_ps.tile([128, 2, CHUNK], F32, tag="blk",
                                      name="pt")
                    pts = pts_pool.tile([128, 2, CHUNK], BF16, tag=ptag)
                    los = []
                    for half, kc in enumerate(pr):
                        los.append(kc * 128 if narrow else 0)
                        for sub in range(kc if narrow else 0, 4):
                            mm(pt[:, half, sub * 128:(sub + 1) * 128],
                               P[:, h, t, sub, kc * 128:(kc + 1) * 128],
                               dfs[sub], start=True, stop=True)
                    pend.append((pr, los, pt, pts))
                    if len(pend) == 2:
                        flush1()
                while pend:
                    flush1()
                if t == nj - 1:
                    ub = o2_pool.tile([128, CHUNK], F32R, tag=f"ub{h}",
                                      name=f"ub{h}")
                    ub_rot.copy(ub, up)
                    ubs[h] = ub

            op_q = []   # deferred out-proj units, spread across later pairs
            ch_q = []   # deferred per-head chain closures

            def outproj(qi, ubs):
                qsl = slice(qi * CHUNK, (qi + 1) * CHUNK)

                def unit(mo):
                    def run():
                        po = misc_ps.tile([128, CHUNK], F32, tag="blk",
                                          name="po")
                        for t in range(HL):
                            mm(po, wo_sb[:, t, mo * 128:(mo + 1) * 128],
                               ubs[t], start=(t == 0), stop=(t == HL - 1))
                        ob = o_pool.tile([128, CHUNK], F32, tag="osb")
                        po_rot.copy(ob, po)
                        nc.sync.dma_start(
                            ap["outT"][mo * 128:(mo + 1) * 128, qsl], ob)
                    return run
                for mo in range(HID // 128):
                    op_q.append(unit(mo))

            # fine-grained interleave, h-major: each p2(qi-1) unit (h,t) is
            # emitted just before the p1(qi) unit (h,t) that overwrites its
            # P slot, so PE transpose/PV work fills the scalar engine's exp
            # latency.  h-major keeps only one PV accumulator (+1 rotation
            # slack) live in PSUM.  diag(G) tiles are built one pair ahead
            # and max-reduces are emitted one pair late to keep the DVE
            # queue free of not-yet-ready work.
            prev = None   # (qi, Gw, ups, ubs)
            df_store = {}
            for qi in range(NQ):
                nj = len(plan[qi])
                njp = len(plan[prev[0]]) if prev is not None else 0
                tl = p1_alloc(qi)
                seq = [(t, h) for h in range(HL) for t in range(nj)]
                for i, (t, h) in enumerate(seq):
                    if prev is not None and t < njp:
                        dfs = df_store.pop((t, h), None)
                        if dfs is None:
                            dfs = build_dfs(prev[1], t, h)
                        p2_unit(prev[0], t, h, prev[2], prev[3], dfs)
                        flush_reduce()
                    if prev is not None:
                        for (t2, h2) in seq[i + 1:i + 2]:
                            if t2 < njp and (t2, h2) not in df_store:
                                df_store[(t2, h2)] = build_dfs(prev[1], t2, h2)
                    p1_unit(qi, t, h, tl)
                    while len(red_q) > 1:
                        red_q.pop(0)()
                    if op_q:
                        op_q.pop(0)()
                    if ch_q:
                        ch_q.pop(0)()
                    if t == nj - 1:
                        # this head's chain closes per-row (the next stage
                        # never waits on a full-width chain), but its DVE
                        # burst is deferred into the next row's pair stream
                        def chain(h=h):
                            flush_reduce(99)
                            p1_chains(qi, tl, slice(h * 4, h * 4 + 4))
                        ch_q.append(chain)
                while ch_q:
                    ch_q.pop(0)()
                if prev is not None:
                    outproj(prev[0], prev[3])
                prev = (qi, tl["Gw"], {}, {})
                df_store = {}
            seq = [(t, h) for h in range(HL) for t in range(NQ)]
            for i, (t, h) in enumerate(seq):
                dfs = df_store.pop((t, h), None)
                if dfs is None:
                    dfs = build_dfs(prev[1], t, h)
                p2_unit(NQ - 1, t, h, prev[2], prev[3], dfs, rot=drain_rot)
                if op_q:
                    op_q.pop(0)()
                for (t2, h2) in seq[i + 1:i + 2]:
                    df_store[(t2, h2)] = build_dfs(prev[1], t2, h2)
            outproj(NQ - 1, prev[3])
            for run in op_q:
                run()


def _build_program(plan):
    nc = bacc.Bacc("TRN2", target_bir_lowering=False, debug=False,
                   enable_asserts=False, num_devices=NCORES)
    ap = {}
    ap["hsT"] = nc.dram_tensor("hsT", [HID, S], BF16, kind="ExternalInput").ap()
    ap["wqk"] = nc.dram_tensor("wqk", [HID, (HL + KVL) * D], BF16, kind="ExternalInput").ap()
    ap["wv"] = nc.dram_tensor("wv", [HID, KVL * D], BF16, kind="ExternalInput").ap()
    ap["wo"] = nc.dram_tensor("wo", [HL * D, HID], F32R, kind="ExternalInput").ap()
    ap["bqk"] = nc.dram_tensor("bqk", [D, HL + KVL], F32, kind="ExternalInput").ap()
    ap["bv"] = nc.dram_tensor("bv", [1, KVL * D], F32R, kind="ExternalInput").ap()
    ap["cosT"] = nc.dram_tensor("cosT", [D, S], F32, kind="ExternalInput").ap()
    ap["sinT"] = nc.dram_tensor("sinT", [D, S], F32, kind="ExternalInput").ap()
    ap["rmat"] = nc.dram_tensor("rmat", [D, D], F32R, kind="ExternalInput").ap()
    ap["imatb"] = nc.dram_tensor("imatb", [128, 128], BF16, kind="ExternalInput").ap()
    ap["maskcb"] = nc.dram_tensor("maskcb", [128, 4, CHUNK], BF16, kind="ExternalInput").ap()
    ap["ones1"] = nc.dram_tensor("ones1", [1, 128], F32R, kind="ExternalInput").ap()
    ap["outT"] = nc.dram_tensor("outT", [HID, S], F32, kind="ExternalOutput").ap()

    with tile.TileContext(nc) as tc:
        _emit(tc, ap, plan)
    nc.compile()
    return nc


def _host_inputs(inputs):
    hs = np.asarray(inputs["hidden_states"], dtype=np.float32)
    Wq = np.asarray(inputs["Wq"], dtype=np.float32)
    bq = np.asarray(inputs["bq"], dtype=np.float32)
    Wk = np.asarray(inputs["Wk"], dtype=np.float32)
    bk = np.asarray(inputs["bk"], dtype=np.float32)
    Wv = np.asarray(inputs["Wv"], dtype=np.float32)
    bv_ = np.asarray(inputs["bv"], dtype=np.float32)
    Wo = np.asarray(inputs["Wo"], dtype=np.float32)

    cosT, sinT = _rope_tables()
    R = np.zeros((D, D), dtype=np.float32)
    R[64 + np.arange(64), np.arange(64)] = -1.0   # out[d'<64] = -q[d'+64]
    R[np.arange(64), 64 + np.arange(64)] = 1.0    # out[d'>=64] = q[d'-64]
    Ib = _bf16(np.eye(128, dtype=np.float32))

    # causal diag-block mask rows: row p of q-sub `sub` vs full chunk cols
    q_idx = np.arange(128)[:, None]
    c_idx = np.arange(CHUNK)[None, :]
    mcb = np.zeros((128, 4, CHUNK), dtype=np.float32)
    for sub in range(4):
        mcb[:, sub, :] = np.where(c_idx <= sub * 128 + q_idx, 0.0, NEG)
    mcb_b = _bf16(mcb)

    Wq4 = (Wq * SCALE).reshape(HID, H, D)
    bq4 = (bq * SCALE).reshape(H, D)
    Wk4 = Wk.reshape(HID, HKV, D)
    bk4 = bk.reshape(HKV, D)
    Wv4 = Wv.reshape(HID, HKV, D)
    bv4 = bv_.reshape(HKV, D)
    Wo4 = Wo.reshape(H, D, HID)

    in_maps = []
    for c in range(NCORES):
        b, hg = divmod(c, NCORES // B)
        qh = slice(hg * HL, (hg + 1) * HL)
        kvh = slice(hg * KVL, (hg + 1) * KVL)
        wqk = np.concatenate([
            Wq4[:, qh].reshape(HID, HL * D),
            Wk4[:, kvh].reshape(HID, KVL * D)], axis=1)
        bqk = np.concatenate([bq4[qh], bk4[kvh]], axis=0).T  # [D, HL+KVL]
        in_maps.append({
            "hsT": _bf16(hs[b].T),
            "wqk": _bf16(wqk),
            "wv": _bf16(Wv4[:, kvh].reshape(HID, KVL * D)),
            "wo": _f32r_round(Wo4[qh].reshape(HL * D, HID)),
            "bqk": np.ascontiguousarray(bqk),
            "bv": _f32r_round(bv4[kvh].reshape(1, KVL * D)),
            "cosT": cosT,
            "sinT": sinT,
            "rmat": R,
            "imatb": Ib,
            "maskcb": mcb_b,
            "ones1": np.ones((1, 128), dtype=np.float32),
        })
    return in_maps


def get_program(inputs):
    am = np.asarray(inputs["attention_mask"], dtype=np.float32)
    plan = _check_causal(am)
    key = "causal"
    if key not in _CACHE:
        _CACHE[key] = _build_program(plan)
    return _CACHE[key], plan, []


def run(inputs, **spmd_kwargs):
    nc, plan, _ = get_program(inputs)
    in_maps = _host_inputs(inputs)
    res = run_bass_kernel_spmd(nc, in_maps, core_ids=list(range(NCORES)),
                               **spmd_kwargs)
    bo = np.asarray(inputs["bo"], dtype=np.float32)
    out = np.empty((B, S, HID), dtype=np.float32)
    gpb = NCORES // B
    for b in range(B):
        acc = np.zeros((HID, S), dtype=np.float32)
        for c in range(b * gpb, (b + 1) * gpb):
            acc += res.results[c]["outT"]
        out[b] = acc.T + bo
    return out, res


def kernel(**inputs) -> np.ndarray:
    out, _ = run(inputs)
    return out


# revision 88
# speedup vs baseline: 1.0010x; 1.0010x over previous
"""Trainium2 Bass kernel for MemoryEfficientFlashAttention (B=2,S=2048,HID=2048,H=16,HKV=8,D=128,CHUNK=512).

Sharding: 8 cores = 2 batches x 4 head-groups (4 q heads / 2 kv heads per core).
Each core computes q/k/v projections (+RoPE), the chunked flash-attention
recurrence, and a row-sharded partial of the output projection (transposed).
Host sums the 4 partials per batch and adds bo.

Math: the reference's scan step is algebraically
    o_j = (o_{j-1} * e^{m_{j-1}} + Y_j) / (e^{m_{j-1}} + S_j)
with Y_j = exp(sc_j - m_j) @ V_j, m_j the running max.  Unrolled:
    o_n = sum_j exp(sc_j - m_j) @ V_j * F_j,
    F_j = prod_{l>=j} d_l^-1 * d_n^-flag,   d_l = e^{m_{l-1}-m_l} + T_l,
where flag=1 iff the globally-last kv chunk was processed (the reference's
final o/d divide; skipped chunks have d=1 exactly under fp32 exp underflow).

Single score pass: scores are computed once ([q-part, k]) and exponentiated
with NO max subtraction (|sc| <~ 10, safe in fp32/bf16): P_t = exp(sc_t),
stored bf16, with T-hat accumulated by the activation.  Chunk maxes come
from bf16 reduces OFF the critical path, and the chains run on
M_t = e^{m_t} (running max of chunk maxes):
    d_t = (M_{t-1} + That_t) / M_t,     G_t = F_t / M_t
so no Ln/exp in the chain at all.  Pass 2 applies G and transposes P in one
PE op per 128-block:
    matmul(lhsT=P_block, rhs=diag(G))  ==  P^T diag(G)
and accumulates u = sum V^T (P^T diag(G)) directly in PSUM.  No score
recompute and no exponent-bias injection matmuls.  Engine placement per the
TRN2 ISA: PSUM evacuation only on DVE/Act, elementwise SBUF chain work on
GpSimd (which cannot touch PSUM or do tensor_tensor max), transposes/PV/
projections on PE.
"""

import os
import sys
from contextlib import ExitStack

import numpy as np
import ml_dtypes

sys.path.insert(0, "/opt/trn_rl_repo")
os.environ.setdefault("MYCRO_LOCAL_CACHE", "1")

import concourse.bass as bass  # noqa: E402
import concourse.tile as tile  # noqa: E402
from concourse import bacc, mybir  # noqa: E402
from concourse.bass_utils import run_bass_kernel_spmd  # noqa: E402

B, S, HID = 2, 2048, 2048
H, HKV, D = 16, 8, 128
CHUNK = 512
THETA = 1000000.0
NCORES = 8
HL = H // (NCORES // B)      # 4 local q heads
KVL = HKV // (NCORES // B)   # 2 local kv heads
NQ = S // CHUNK              # 4 chunks
NT = HID // 128              # 16 hid tiles
SCALE = 1.0 / np.sqrt(np.float32(D))
NEG = -1e9

F32 = mybir.dt.float32
F32R = mybir.dt.float32r
BF16 = mybir.dt.bfloat16
Alu = mybir.AluOpType
Act = mybir.ActivationFunctionType
AxX = mybir.AxisListType.X

_CACHE = {}


def _f32r_round(a):
    """Round fp32 to the fp32r format (1s/8e/11m in the high 20 bits):
    round-to-nearest-even at mantissa bit 12."""
    u = np.ascontiguousarray(a, dtype=np.float32).view(np.uint32).copy()
    low = u & np.uint32(0xFFF)
    base = u & ~np.uint32(0xFFF)
    lsb = (base >> 12) & np.uint32(1)
    round_up = (low > 0x800) | ((low == 0x800) & (lsb == 1))
    out = base + (round_up.astype(np.uint32) << 12)
    return out.view(np.float32)


def _bf16(a):
    return np.asarray(a, dtype=ml_dtypes.bfloat16)


def _rope_tables():
    inv_freq = 1.0 / (THETA ** (np.arange(0, D, 2, dtype=np.float32) / D))
    pos = np.arange(S, dtype=np.float32)
    freqs = pos[:, None].astype(np.float32) * inv_freq[None, :]
    emb = np.concatenate([freqs, freqs], axis=-1)  # [S, D]
    cosT = np.cos(emb).astype(np.float32).T.copy()
    sinT = np.sin(emb).astype(np.float32).T.copy()
    return cosT, sinT  # [D, S]


def _check_causal(am):
    """Assert the mask is the canonical causal mask (shared across batches)
    and return the per-qi chunk plan [(j, is_diag), ...]."""
    tri = np.tril(np.ones((S, S), dtype=bool))
    want = np.where(tri, 0.0, NEG).astype(np.float32)
    for b in range(am.shape[0]):
        assert np.array_equal(am[b, 0], want), "kernel specialized for causal mask"
    plan = {}
    for qi in range(NQ):
        plan[qi] = [(j, j == qi) for j in range(qi + 1)]
    return plan


class _Rot:
    """Round-robin engine selector for PSUM->SBUF copies."""

    def __init__(self, nc, engines):
        self.ops = []
        for e in engines:
            if e == "v":
                self.ops.append(lambda o, i: nc.vector.tensor_copy(o, i))
            elif e == "g":
                self.ops.append(lambda o, i: nc.gpsimd.tensor_copy(o, i))
            else:
                self.ops.append(lambda o, i: nc.scalar.copy(o, i))
        self.i = 0

    def copy(self, out, in_):
        self.ops[self.i % len(self.ops)](out, in_)
        self.i += 1


def _emit(tc, ap, plan):
    nc = tc.nc

    def mm(out, lhsT, rhs, start, stop):
        nc.tensor.matmul(out, lhsT, rhs, start=start, stop=stop)

    with ExitStack() as top:
        # ---------------- persistent tensors ----------------
        pers = top.enter_context(tc.tile_pool(name="pers", bufs=1))
        QT = pers.tile([128, HL, S], F32R)             # rope'd q^T  [d, h, s]
        KT = pers.tile([128, KVL, S], F32R)            # rope'd k^T  [d, kv, s]
        V = pers.tile([128, S // 128, KVL * D], BF16)  # v natural [s_p, s_t, kv*d]
        I128b = pers.tile([128, 128], BF16)
        maskcb = pers.tile([128, 4, CHUNK], BF16)      # causal diag-block mask rows
        ones1 = pers.tile([1, 128], F32R)
        R128 = pers.tile([128, 128], F32R)
        bqk = pers.tile([128, HL + KVL], F32)
        bv = pers.tile([1, KVL * D], F32R)

        # ---------------- phase A: projections + rope ----------------
        with ExitStack() as ph1:
            w_pool = ph1.enter_context(tc.tile_pool(name="wres", bufs=1))
            xt_pool = ph1.enter_context(tc.tile_pool(name="xt", bufs=2))
            cs_pool = ph1.enter_context(tc.tile_pool(name="cs", bufs=2))
            raw_pool = ph1.enter_context(tc.tile_pool(name="raw", bufs=2))
            t_pool = ph1.enter_context(tc.tile_pool(name="ropetmp", bufs=2))
            psP = ph1.enter_context(tc.tile_pool(name="psP", bufs=6, space="PSUM"))
            psR = ph1.enter_context(tc.tile_pool(name="psR", bufs=1, space="PSUM"))
            psV = ph1.enter_context(tc.tile_pool(name="psV", bufs=1, space="PSUM"))

            wqk_r = ap["wqk"].rearrange("(t p) m -> p t m", p=128)
            wqk_sb = w_pool.tile([128, NT, (HL + KVL) * D], BF16)
            wv_sb = w_pool.tile([128, NT, KVL * D], BF16)
            hsT_r = ap["hsT"].rearrange("(t p) s -> p t s", p=128)

            for sq in range(NQ):
                ssl = slice(sq * CHUNK, (sq + 1) * CHUNK)
                xt = xt_pool.tile([128, NT, CHUNK], BF16)
                cost = cs_pool.tile([128, CHUNK], F32, tag="cos")
                sint = cs_pool.tile([128, CHUNK], F32, tag="sin")
                if sq == 0:
                    # DMA order tuned so the first matmuls start ~1.5us in:
                    # weight/x tiles stream in consumption order, consts
                    # needed later go last.
                    nc.sync.dma_start(R128, ap["rmat"])
                    nc.sync.dma_start(bqk, ap["bqk"])
                    for t in range(4):
                        nc.sync.dma_start(wqk_sb[:, t], wqk_r[:, t])
                        nc.sync.dma_start(xt[:, t], hsT_r[:, t, ssl])
                    nc.sync.dma_start(cost, ap["cosT"][:, ssl])
                    nc.sync.dma_start(sint, ap["sinT"][:, ssl])
                    for t in range(4, NT):
                        nc.sync.dma_start(wqk_sb[:, t], wqk_r[:, t])
                        nc.sync.dma_start(xt[:, t], hsT_r[:, t, ssl])
                    nc.sync.dma_start(bv, ap["bv"])
                    nc.sync.dma_start(ones1, ap["ones1"])
                    nc.sync.dma_start(
                        wv_sb, ap["wv"].rearrange("(t p) m -> p t m", p=128))
                    nc.sync.dma_start(I128b, ap["imatb"])
                    nc.sync.dma_start(maskcb, ap["maskcb"])
                else:
                    for t in range(NT):
                        nc.sync.dma_start(xt[:, t], hsT_r[:, t, ssl])
                    nc.sync.dma_start(cost, ap["cosT"][:, ssl])
                    nc.sync.dma_start(sint, ap["sinT"][:, ssl])

                # q^T and k^T projections, rope'd.  sq==0 runs t-major so
                # each x tile is consumed as its DMA lands (6 open groups).
                if sq == 0:
                    pss = [psP.tile([128, CHUNK], F32, tag="sc", name=f"ps{m}")
                           for m in range(HL + KVL)]
                    for t in range(NT):
                        for m in range(HL + KVL):
                            mm(pss[m], wqk_sb[:, t, m * 128:(m + 1) * 128],
                               xt[:, t], start=(t == 0), stop=(t == NT - 1))
                for m in range(HL + KVL):
                    if sq == 0:
                        ps = pss[m]
                    else:
                        ps = psP.tile([128, CHUNK], F32, tag="sc", name="ps")
                        for t in range(NT):
                            mm(ps, wqk_sb[:, t, m * 128:(m + 1) * 128],
                               xt[:, t], start=(t == 0), stop=(t == NT - 1))
                    raw = raw_pool.tile([128, CHUNK], F32R)
                    nc.vector.tensor_scalar_add(raw, ps, bqk[:, m:m + 1])
                    pr = psR.tile([128, CHUNK], F32)
                    mm(pr, R128, raw, start=True, stop=True)
                    t1 = t_pool.tile([128, CHUNK], F32, tag="t1")
                    nc.gpsimd.tensor_mul(t1, raw.bitcast(F32), cost)
                    t2 = t_pool.tile([128, CHUNK], F32, tag="t2")
                    nc.vector.tensor_mul(t2, pr, sint)
                    dest = QT[:, m, ssl] if m < HL else KT[:, m - HL, ssl]
                    nc.vector.tensor_add(dest, t1, t2)

                # v projection (natural layout), bias via K=1 matmul
                for ss in range(CHUNK // 128):
                    pv = psV.tile([128, KVL * D], F32)
                    for t in range(NT):
                        mm(pv, xt[:, t, ss * 128:(ss + 1) * 128], wv_sb[:, t],
                           start=(t == 0), stop=False)
                    mm(pv, ones1, bv, start=False, stop=True)
                    (nc.vector.tensor_copy if ss % 2 else nc.scalar.copy)(V[:, sq * 4 + ss, :], pv)

        # ---------------- phase B: attention + out-proj ----------------
        with ExitStack() as ph2:
            pb_pool = ph2.enter_context(tc.tile_pool(name="probs", bufs=1))
            wo_pool = ph2.enter_context(tc.tile_pool(name="wo", bufs=1))
            ch_pool = ph2.enter_context(tc.tile_pool(name="chain", bufs=2))
            df_pool = ph2.enter_context(tc.tile_pool(name="dfp", bufs=12))
            pts_pool = ph2.enter_context(tc.tile_pool(name="pts", bufs=4))
            mh_pool = ph2.enter_context(tc.tile_pool(name="mh", bufs=2))
            o2_pool = ph2.enter_context(tc.tile_pool(name="uout", bufs=2))
            o_pool = ph2.enter_context(tc.tile_pool(name="osb", bufs=4))
            sc_ps = ph2.enter_context(tc.tile_pool(name="scps", bufs=3, space="PSUM"))
            u_ps = ph2.enter_context(tc.tile_pool(name="ups", bufs=1, space="PSUM"))
            misc_ps = ph2.enter_context(tc.tile_pool(name="mps", bufs=2, space="PSUM"))

            # bf16 probabilities, slot per (h, chunk-idx, q-sub)
            P = pb_pool.tile([128, HL, NQ, 4, CHUNK], BF16)
            wo_sb = wo_pool.tile([128, HL, HID], F32R)
            nc.sync.dma_start(wo_sb, ap["wo"].rearrange("(t p) m -> p t m", p=128))

            pt_rot = _Rot(nc, ["v", "s", "v"])
            drain_rot = _Rot(nc, ["s", "v"])
            ub_rot = _Rot(nc, ["v"])
            po_rot = _Rot(nc, ["s", "v"])

            # P_t = exp(sc_t) (no max subtraction: |sc| <~ 10 so exp is
            # safe in fp32/bf16).  Chunk maxes come from a single bf16
            # reduce per (h,t) OFF the critical path; the chains work on
            # M_t = e^{m_t} = running max of chunk maxes:
            #   d_t = (M_{t-1} + That_t) / M_t      (M_{-1} = 0)
            #   G_t = (prod_{l>=t} d_l^-1) * d_n^-flag / M_t
            # and the stored P gets weight P*G via diag(G) in pass 2.

            def p1_alloc(qi):
                tl = {n: ch_pool.tile([128, NQ, HL * 4], F32, tag=n, name=n)
                      for n in ("Ts", "Mr", "Sm", "rr", "Fw", "Gw")}
                tl["mx"] = ch_pool.tile([128, NQ, HL * 4, 8], F32, tag="mx",
                                        name="mx")
                return tl

            red_q = []   # deferred max-reduces: emitted one unit later so
                         # the DVE queue never head-of-line blocks on exps
            red_cnt = [0]

            def flush_reduce(n=1):
                for _ in range(min(n, len(red_q))):
                    red_q.pop(0)()

            def p1_unit(qi, t, h, tl):
                j, diag = plan[qi][t]
                narrow = diag and qi != 0
                for sub in range(4):
                    col = h * 4 + sub
                    w = (sub + 1) * 128 if narrow else CHUNK
                    q0 = qi * CHUNK + sub * 128
                    k0 = j * CHUNK
                    ps = sc_ps.tile([128, CHUNK], F32, tag="sc")
                    mm(ps[:, :w], QT[:, h, q0:q0 + 128],
                       KT[:, h // 2, k0:k0 + w],
                       start=True, stop=not diag)
                    if diag:
                        mm(ps[:, :w], I128b, maskcb[:, sub, :w],
                           start=False, stop=True)
                    nc.scalar.activation(
                        P[:, h, t, sub, :w], ps[:, :w], Act.Exp,
                        accum_out=tl["Ts"][:, t, col:col + 1])

                def reduce():
                    if narrow:
                        for sub in range(4):
                            col = h * 4 + sub
                            nc.vector.tensor_reduce(
                                tl["mx"][:, t, col, 0:1],
                                P[:, h, t, sub, :(sub + 1) * 128],
                                axis=AxX, op=Alu.max)
                    else:
                        # 2-level bf16 max tree on DVE (2x mode) + reduce
                        mh = mh_pool.tile([128, 4, CHUNK // 2], BF16,
                                          tag="mh")
                        nc.vector.tensor_tensor(
                            mh, P[:, h, t, :, :CHUNK // 2],
                            P[:, h, t, :, CHUNK // 2:], Alu.max)
                        mq = mh_pool.tile([128, 4, CHUNK // 4], BF16,
                                          tag="mq")
                        nc.vector.tensor_tensor(
                            mq, mh[:, :, :CHUNK // 4], mh[:, :, CHUNK // 4:],
                            Alu.max)
                        nc.vector.tensor_reduce(
                            tl["mx"][:, t, h * 4:h * 4 + 4, 0:1], mq,
                            axis=AxX, op=Alu.max)
                red_q.append(reduce)

            def p1_chains(qi, tl, cs=slice(0, HL * 4)):
                chunks = plan[qi]
                nj = len(chunks)
                Ts, mx, Mr, Sm, rr, Fw, Gw = (tl[n] for n in
                                              ("Ts", "mx", "Mr", "Sm", "rr",
                                               "Fw", "Gw"))
                nc.gpsimd.tensor_copy(Mr[:, 0, cs], mx[:, 0, cs, 0])
                for t in range(1, nj):
                    nc.vector.tensor_tensor(Mr[:, t, cs], Mr[:, t - 1, cs],
                                            mx[:, t, cs, 0], Alu.max)
                nc.gpsimd.tensor_copy(Sm[:, 0, cs], Ts[:, 0, cs])
                for t in range(1, nj):
                    nc.gpsimd.tensor_add(Sm[:, t, cs], Mr[:, t - 1, cs],
                                         Ts[:, t, cs])
                nc.vector.reciprocal(rr[:, :nj, cs], Sm[:, :nj, cs])
                nc.gpsimd.tensor_mul(rr[:, :nj, cs], rr[:, :nj, cs],
                                     Mr[:, :nj, cs])   # r_t = M_t / S_t
                if any(j == NQ - 1 for (j, _) in chunks):
                    nc.gpsimd.tensor_mul(rr[:, nj - 1, cs], rr[:, nj - 1, cs],
                                         rr[:, nj - 1, cs])
                nc.gpsimd.tensor_copy(Fw[:, nj - 1, cs], rr[:, nj - 1, cs])
                for t in range(nj - 2, -1, -1):
                    nc.gpsimd.tensor_mul(Fw[:, t, cs], Fw[:, t + 1, cs],
                                         rr[:, t, cs])
                nc.vector.reciprocal(Sm[:, :nj, cs], Mr[:, :nj, cs])
                nc.gpsimd.tensor_mul(Gw[:, :nj, cs], Fw[:, :nj, cs],
                                     Sm[:, :nj, cs])

            def build_dfs(Gw, t, h):
                dfs = []
                for sub in range(4):
                    df = df_pool.tile([128, 128], BF16, tag="df")
                    nc.gpsimd.tensor_scalar_mul(
                        df, I128b, Gw[:, t, h * 4 + sub:h * 4 + sub + 1])
                    dfs.append(df)
                return dfs

            def p2_unit(qi, t, h, ups, ubs, dfs, rot=None, utag="u",
                        ptag="pts"):
                # PV group per h: start at (t=0, kc=0) full-width; the diag
                # chunk (t=nj-1) runs kc DESC so the group's stop lands on a
                # full-width op (kc=0), keeping psum zero-region bookkeeping
                # clean.  qi==0's single chunk is full-width throughout.
                # The kc loop is software-pipelined: the next kc's transposes
                # are emitted before this kc's PV so PE never waits on the
                # psum->sbuf copy.
                rot = rot or pt_rot
                chunks = plan[qi]
                nj = len(chunks)
                j, diag = chunks[t]
                narrow = diag and qi != 0
                if t == 0:
                    ups[h] = u_ps.tile([128, CHUNK], F32, tag=utag, name="up")
                up = ups[h]

                def pv(kc, lo, pts):
                    mm(up[:, lo:],
                       V[:, j * 4 + kc, (h // 2) * D:(h // 2 + 1) * D],
                       pts[:, lo:], start=(t == 0 and kc == 0),
                       stop=(t == nj - 1 and kc == (0 if narrow else 3)))

                # kc pairs share one 2-bank psum tile and ONE evacuation
                # copy, halving copy-instruction count
                kprs = [(3, 2), (1, 0)] if narrow else [(0, 1), (2, 3)]
                pend = []

                def flush1():
                    pr, los, pt2, pts2 = pend.pop(0)
                    mlo = min(los)
                    rot.copy(pts2[:, :, mlo:], pt2[:, :, mlo:])
                    for half, kc in enumerate(pr):
                        pv(kc, los[half], pts2[:, half])

                for pr in kprs:
                    pt = misc_ps.tile([128, 2, CHUNK], F32, tag="blk",
                                      name="pt")
                    pts = pts_pool.tile([128, 2, CHUNK], BF16, tag=ptag)
                    los = []
                    for half, kc in enumerate(pr):
                        los.append(kc * 128 if narrow else 0)
                        for sub in range(kc if narrow else 0, 4):
                            mm(pt[:, half, sub * 128:(sub + 1) * 128],
                               P[:, h, t, sub, kc * 128:(kc + 1) * 128],
                               dfs[sub], start=True, stop=True)
                    pend.append((pr, los, pt, pts))
                    if len(pend) == 2:
                        flush1()
                while pend:
                    flush1()
                if t == nj - 1:
                    ub = o2_pool.tile([128, CHUNK], F32R, tag=f"ub{h}",
                                      name=f"ub{h}")
                    ub_rot.copy(ub, up)
                    ubs[h] = ub

            op_q = []   # deferred out-proj units, spread across later pairs
            ch_q = []   # deferred per-head chain closures

            def outproj(qi, ubs):
                qsl = slice(qi * CHUNK, (qi + 1) * CHUNK)

                def unit(mo):
                    def run():
                        po = misc_ps.tile([128, CHUNK], F32, tag="blk",
                                          name="po")
                        for t in range(HL):
                            mm(po, wo_sb[:, t, mo * 128:(mo + 1) * 128],
                               ubs[t], start=(t == 0), stop=(t == HL - 1))
                        ob = o_pool.tile([128, CHUNK], F32, tag="osb")
                        po_rot.copy(ob, po)
                        nc.sync.dma_start(
                            ap["outT"][mo * 128:(mo + 1) * 128, qsl], ob)
                    return run
                for mo in range(HID // 128):
                    op_q.append(unit(mo))

            # fine-grained interleave, h-major: each p2(qi-1) unit (h,t) is
            # emitted just before the p1(qi) unit (h,t) that overwrites its
            # P slot, so PE transpose/PV work fills the scalar engine's exp
            # latency.  h-major keeps only one PV accumulator (+1 rotation
            # slack) live in PSUM.  diag(G) tiles are built one pair ahead
            # and max-reduces are emitted one pair late to keep the DVE
            # queue free of not-yet-ready work.
            prev = None   # (qi, Gw, ups, ubs)
            df_store = {}
            for qi in range(NQ):
                nj = len(plan[qi])
                njp = len(plan[prev[0]]) if prev is not None else 0
                tl = p1_alloc(qi)
                seq = [(t, h) for h in range(HL) for t in range(nj)]
                for i, (t, h) in enumerate(seq):
                    if prev is not None and t < njp:
                        dfs = df_store.pop((t, h), None)
                        if dfs is None:
                            dfs = build_dfs(prev[1], t, h)
                        flush_reduce()
                        p2_unit(prev[0], t, h, prev[2], prev[3], dfs)
                    if prev is not None:
                        for (t2, h2) in seq[i + 1:i + 2]:
                            if t2 < njp and (t2, h2) not in df_store:
                                df_store[(t2, h2)] = build_dfs(prev[1], t2, h2)
                    p1_unit(qi, t, h, tl)
                    while len(red_q) > 1:
                        red_q.pop(0)()
                    if op_q:
                        op_q.pop(0)()
                    if ch_q:
                        ch_q.pop(0)()
                    if t == nj - 1:
                        # this head's chain closes per-row (the next stage
                        # never waits on a full-width chain), but its DVE
                        # burst is deferred into the next row's pair stream
                        def chain(h=h):
                            flush_reduce(99)
                            p1_chains(qi, tl, slice(h * 4, h * 4 + 4))
                        ch_q.append(chain)
                while ch_q:
                    ch_q.pop(0)()
                if prev is not None:
                    outproj(prev[0], prev[3])
                prev = (qi, tl["Gw"], {}, {})
                df_store = {}
            seq = [(t, h) for h in range(HL) for t in range(NQ)]
            for i, (t, h) in enumerate(seq):
                dfs = df_store.pop((t, h), None)
                if dfs is None:
                    dfs = build_dfs(prev[1], t, h)
                p2_unit(NQ - 1, t, h, prev[2], prev[3], dfs, rot=drain_rot)
                if op_q:
                    op_q.pop(0)()
                for (t2, h2) in seq[i + 1:i + 2]:
                    df_store[(t2, h2)] = build_dfs(prev[1], t2, h2)
            outproj(NQ - 1, prev[3])
            for run in op_q:
                run()


def _build_program(plan):
    nc = bacc.Bacc("TRN2", target_bir_lowering=False, debug=False,
                   enable_asserts=False, num_devices=NCORES)
    ap = {}
    ap["hsT"] = nc.dram_tensor("hsT", [HID, S], BF16, kind="ExternalInput").ap()
    ap["wqk"] = nc.dram_tensor("wqk", [HID, (HL + KVL) * D], BF16, kind="ExternalInput").ap()
    ap["wv"] = nc.dram_tensor("wv", [HID, KVL * D], BF16, kind="ExternalInput").ap()
    ap["wo"] = nc.dram_tensor("wo", [HL * D, HID], F32R, kind="ExternalInput").ap()
    ap["bqk"] = nc.dram_tensor("bqk", [D, HL + KVL], F32, kind="ExternalInput").ap()
    ap["bv"] = nc.dram_tensor("bv", [1, KVL * D], F32R, kind="ExternalInput").ap()
    ap["cosT"] = nc.dram_tensor("cosT", [D, S], F32, kind="ExternalInput").ap()
    ap["sinT"] = nc.dram_tensor("sinT", [D, S], F32, kind="ExternalInput").ap()
    ap["rmat"] = nc.dram_tensor("rmat", [D, D], F32R, kind="ExternalInput").ap()
    ap["imatb"] = nc.dram_tensor("imatb", [128, 128], BF16, kind="ExternalInput").ap()
    ap["maskcb"] = nc.dram_tensor("maskcb", [128, 4, CHUNK], BF16, kind="ExternalInput").ap()
    ap["ones1"] = nc.dram_tensor("ones1", [1, 128], F32R, kind="ExternalInput").ap()
    ap["outT"] = nc.dram_tensor("outT", [HID, S], F32, kind="ExternalOutput").ap()

    with tile.TileContext(nc) as tc:
        _emit(tc, ap, plan)
    nc.compile()
    return nc


def _host_inputs(inputs):
    hs = np.asarray(inputs["hidden_states"], dtype=np.float32)
    Wq = np.asarray(inputs["Wq"], dtype=np.float32)
    bq = np.asarray(inputs["bq"], dtype=np.float32)
    Wk = np.asarray(inputs["Wk"], dtype=np.float32)
    bk = np.asarray(inputs["bk"], dtype=np.float32)
    Wv = np.asarray(inputs["Wv"], dtype=np.float32)
    bv_ = np.asarray(inputs["bv"], dtype=np.float32)
    Wo = np.asarray(inputs["Wo"], dtype=np.float32)

    cosT, sinT = _rope_tables()
    R = np.zeros((D, D), dtype=np.float32)
    R[64 + np.arange(64), np.arange(64)] = -1.0   # out[d'<64] = -q[d'+64]
    R[np.arange(64), 64 + np.arange(64)] = 1.0    # out[d'>=64] = q[d'-64]
    Ib = _bf16(np.eye(128, dtype=np.float32))

    # causal diag-block mask rows: row p of q-sub `sub` vs full chunk cols
    q_idx = np.arange(128)[:, None]
    c_idx = np.arange(CHUNK)[None, :]
    mcb = np.zeros((128, 4, CHUNK), dtype=np.float32)
    for sub in range(4):
        mcb[:, sub, :] = np.where(c_idx <= sub * 128 + q_idx, 0.0, NEG)
    mcb_b = _bf16(mcb)

    Wq4 = (Wq * SCALE).reshape(HID, H, D)
    bq4 = (bq * SCALE).reshape(H, D)
    Wk4 = Wk.reshape(HID, HKV, D)
    bk4 = bk.reshape(HKV, D)
    Wv4 = Wv.reshape(HID, HKV, D)
    bv4 = bv_.reshape(HKV, D)
    Wo4 = Wo.reshape(H, D, HID)

    in_maps = []
    for c in range(NCORES):
        b, hg = divmod(c, NCORES // B)
        qh = slice(hg * HL, (hg + 1) * HL)
        kvh = slice(hg * KVL, (hg + 1) * KVL)
        wqk = np.concatenate([
            Wq4[:, qh].reshape(HID, HL * D),
            Wk4[:, kvh].reshape(HID, KVL * D)], axis=1)
        bqk = np.concatenate([bq4[qh], bk4[kvh]], axis=0).T  # [D, HL+KVL]
        in_maps.append({
            "hsT": _bf16(hs[b].T),
            "wqk": _bf16(wqk),
            "wv": _bf16(Wv4[:, kvh].reshape(HID, KVL * D)),
            "wo": _f32r_round(Wo4[qh].reshape(HL * D, HID)),
            "bqk": np.ascontiguousarray(bqk),
            "bv": _f32r_round(bv4[kvh].reshape(1, KVL * D)),
            "cosT": cosT,
            "sinT": sinT,
            "rmat": R,
            "imatb": Ib,
            "maskcb": mcb_b,
            "ones1": np.ones((1, 128), dtype=np.float32),
        })
    return in_maps


def get_program(inputs):
    am = np.asarray(inputs["attention_mask"], dtype=np.float32)
    plan = _check_causal(am)
    key = "causal"
    if key not in _CACHE:
        _CACHE[key] = _build_program(plan)
    return _CACHE[key], plan, []


def run(inputs, **spmd_kwargs):
    nc, plan, _ = get_program(inputs)
    in_maps = _host_inputs(inputs)
    res = run_bass_kernel_spmd(nc, in_maps, core_ids=list(range(NCORES)),
                               **spmd_kwargs)
    bo = np.asarray(inputs["bo"], dtype=np.float32)
    out = np.empty((B, S, HID), dtype=np.float32)
    gpb = NCORES // B
    for b in range(B):
        acc = np.zeros((HID, S), dtype=np.float32)
        for c in range(b * gpb, (b + 1) * gpb):
            acc += res.results[c]["outT"]
        out[b] = acc.T + bo
    return out, res


def kernel(**inputs) -> np.ndarray:
    out, _ = run(inputs)
    return out
